# revision 12
# baseline (speedup 1.0000x reference)
"""Trainium2 Bass kernel for nn_ApproximationLayer (mute-MSB on a row/col grid).

The oracle (reference.py run on the neuron/axon jax backend of this container)
computes, for masked grid cells (r in rows, c in cols):

    y = RN_f32(x * c), c = (2^20+1) * 2^-149   if x < -0.5 (strictly)
    y = x                                       otherwise

(The jnp.frexp lowering on this backend value-converts x to int32 with
round-half-even saturation instead of bitcasting, so the exponent comes out
as 255 exactly when rint(x) <= -1, i.e. x < -0.5; exp2(-129.0) returns the
denormal 0x00400001 = (2^20+1)*2^-149 — one ulp above 2^-129; the where()
multiply is IEEE round-to-nearest-even including denormal results.)

Device strategy (8 NeuronCores, data-parallel over batch; 32 planes/core):
  - Only rows in `rows` can change. Bulk-copy all other rows DRAM->DRAM at
    full DMA rate (no compute engine touches ~94% of the data).
  - For masked rows, stream [128, 1024] f32 tiles through SBUF:
      mask  = x <  colthr        (colthr = -0.5 on selected cols, -FLT_MAX off)
      muted = (x * 2^-100) * ((2^20+1) * 2^-49)   # == RN(x*c), both steps on DVE
      x     = where(mask, muted, x)                # copy_predicated
    DVE denormal *results* are IEEE-exact on TRN2 (verified incl. ties);
    denormal *operands* are not, hence the two-step multiply.
"""
import sys

sys.path.insert(0, "/opt/trn_rl_repo")

import numpy as np

import concourse.bass as bass
import concourse.tile as tile
from concourse import bacc, mybir
from concourse.bass_utils import run_bass_kernel_spmd

B, R, C = 256, 1024, 1024
N_CORES = 8
BS = B // N_CORES  # planes per core

F32 = mybir.dt.float32
LT = mybir.AluOpType.is_lt
MULT = mybir.AluOpType.mult

_S1 = float(2.0 ** -100)
_S2 = float((2 ** 20 + 1) * 2.0 ** -49)
_NEG_FLT_MAX = -3.4028234663852886e38


def _runs(sorted_vals):
    """Contiguous runs [(start, len), ...] of a sorted unique int array."""
    runs = []
    for v in sorted_vals:
        if runs and v == runs[-1][0] + runs[-1][1]:
            runs[-1][1] += 1
        else:
            runs.append([int(v), 1])
    return [(s, n) for s, n in runs]


def _build(row_runs, have_patch):
    """Build the per-core Bacc graph, specialized on the masked-row layout."""
    nc = bacc.Bacc("TRN2", target_bir_lowering=False, debug=False, num_devices=N_CORES)
    x = nc.dram_tensor("x", [BS, R, C], F32, kind="ExternalInput").ap()
    colthr = nc.dram_tensor("colthr", [128, 8 * C], F32, kind="ExternalInput").ap()
    out = nc.dram_tensor("out", [BS, R, C], F32, kind="ExternalOutput").ap()

    # complement of masked rows -> bulk DRAM->DRAM copy ranges
    bulk_ranges = []
    pos = 0
    for s, n in row_runs:
        if s > pos:
            bulk_ranges.append((pos, s))
        pos = s + n
    if pos < R:
        bulk_ranges.append((pos, R))

    # segments: (plane, row_start, n_rows) with n_rows <= 128, packed into tiles
    segments = []
    for p in range(BS):
        for s, n in row_runs:
            o = 0
            while o < n:
                k = min(128, n - o)
                segments.append((p, s + o, k))
                o += k
    tiles = []  # list of lists of (plane, row_start, n_rows, part_ofs)
    cur, used = [], 0
    for p, s, n in segments:
        if used + n > 128:
            tiles.append(cur)
            cur, used = [], 0
        cur.append((p, s, n, used))
        used += n
    if cur:
        tiles.append(cur)

    # Fast path: a single run of masked rows (the spec's arange case) -> tile
    # partitions = masked rows, free dim = planes x cols ("p r c -> r p c"),
    # so the whole patch is a handful of big DMAs (3-dim APs).
    PPC = 4  # planes per patch chunk
    fast = (
        have_patch
        and len(row_runs) == 1
        and row_runs[0][1] <= 128
        and BS % PPC == 0
    )

    with tile.TileContext(nc) as tc:
        with tc.tile_pool(name="cmask", bufs=1) as cmask_pool, \
             tc.tile_pool(name="xin", bufs=2) as xin_pool, \
             tc.tile_pool(name="tmp", bufs=2) as tmp_pool:
            # Patch path rides the ACT HWDGE ring (nc.scalar) so it is not
            # FIFO-ordered behind the bulk copy on the SP ring (nc.sync).
            if fast:
                s0, nrow = row_runs[0]
                W = PPC * C
                cm = cmask_pool.tile([nrow, W], F32)
                nc.scalar.dma_start(cm[:], colthr[:nrow, :W])
                for b in range(0, BS, PPC):
                    t = xin_pool.tile([nrow, W], F32)
                    tv = t[:].rearrange("r (p c) -> r p c", p=PPC)
                    src = x[b:b + PPC, s0:s0 + nrow, :].rearrange("p r c -> r p c")
                    nc.scalar.dma_start(tv, src)
                    mask = tmp_pool.tile([nrow, W], mybir.dt.int32, tag="mask")
                    nc.vector.tensor_tensor(mask[:], t[:], cm[:], LT)
                    muted = tmp_pool.tile([nrow, W], F32, tag="muted")
                    nc.vector.tensor_scalar(muted[:], t[:], _S1, _S2, MULT, MULT)
                    nc.vector.copy_predicated(t[:], mask[:], muted[:])
                    dst = out[b:b + PPC, s0:s0 + nrow, :].rearrange("p r c -> r p c")
                    nc.scalar.dma_start(dst, t[:].rearrange("r (p c) -> r p c", p=PPC))
            elif have_patch:
                cm = cmask_pool.tile([128, C], F32)
                nc.scalar.dma_start(cm[:], colthr[:, :])
                for segs in tiles:
                    t = xin_pool.tile([128, C], F32)
                    for p, s, n, ofs in segs:
                        nc.scalar.dma_start(t[ofs:ofs + n, :], x[p, s:s + n, :])
                    mask = tmp_pool.tile([128, C], mybir.dt.int32, tag="mask")
                    nc.vector.tensor_tensor(mask[:], t[:], cm[:], LT)
                    muted = tmp_pool.tile([128, C], F32, tag="muted")
                    nc.vector.tensor_scalar(muted[:], t[:], _S1, _S2, MULT, MULT)
                    nc.vector.copy_predicated(t[:], mask[:], muted[:])
                    for p, s, n, ofs in segs:
                        nc.scalar.dma_start(out[p, s:s + n, :], t[ofs:ofs + n, :])

            # Bulk: contiguous DRAM->DRAM copies, 4 planes per dma_start.
            for p in range(0, BS, 4):
                for r0, r1 in bulk_ranges:
                    nc.sync.dma_start(out[p:p + 4, r0:r1, :], x[p:p + 4, r0:r1, :])
    nc.compile()
    return nc


def kernel(x, rows, cols):
    x = np.ascontiguousarray(np.asarray(x, dtype=np.float32))
    rows = np.asarray(rows)
    cols = np.asarray(cols)
    assert x.shape == (B, R, C)

    rows_u = np.unique(rows.astype(np.int64))
    cols_u = np.unique(cols.astype(np.int64))
    have_patch = len(rows_u) > 0 and len(cols_u) > 0
    row_runs = _runs(rows_u) if have_patch else []

    colthr_vec = np.full((C,), _NEG_FLT_MAX, dtype=np.float64)
    if have_patch:
        colthr_vec[cols_u] = -0.5
    # replicated wide enough for the fast path's multi-plane free dim
    colthr = np.tile(colthr_vec.astype(np.float32)[None, :], (128, 8)).copy()

    nc = _build(row_runs, have_patch)

    in_maps = [
        {"x": x[i * BS:(i + 1) * BS], "colthr": colthr} for i in range(N_CORES)
    ]
    res = run_bass_kernel_spmd(nc, in_maps, core_ids=list(range(N_CORES)))
    kernel.last_result = res

    out = np.empty((B, R, C), dtype=np.float32)
    for i in range(N_CORES):
        out[i * BS:(i + 1) * BS] = res.results[i]["out"]
    return out


kernel.last_result = None


# revision 15
# speedup vs baseline: 1.6818x; 1.6818x over previous
"""Trainium2 Bass kernel for nn_ApproximationLayer (mute-MSB on a row/col grid).

The oracle (reference.py run on the neuron/axon jax backend of this container)
computes, for masked grid cells (r in rows, c in cols):

    y = RN_f32(x * c), c = (2^20+1) * 2^-149   if x < -0.5 (strictly)
    y = x                                       otherwise

(The jnp.frexp lowering on this backend value-converts x to int32 with
round-half-even saturation instead of bitcasting, so the exponent comes out
as 255 exactly when rint(x) <= -1, i.e. x < -0.5; exp2(-129.0) returns the
denormal 0x00400001 = (2^20+1)*2^-149 — one ulp above 2^-129; the where()
multiply is IEEE round-to-nearest-even including denormal results.)

Device strategy (8 NeuronCores, data-parallel over batch; 32 planes/core):
  - Only rows in `rows` can change. Bulk-copy all other rows DRAM->DRAM at
    full DMA rate (no compute engine touches ~94% of the data).
  - For masked rows, stream [128, 1024] f32 tiles through SBUF:
      mask  = x <  colthr        (colthr = -0.5 on selected cols, -FLT_MAX off)
      muted = (x * 2^-100) * ((2^20+1) * 2^-49)   # == RN(x*c), both steps on DVE
      x     = where(mask, muted, x)                # copy_predicated
    DVE denormal *results* are IEEE-exact on TRN2 (verified incl. ties);
    denormal *operands* are not, hence the two-step multiply.
"""
import sys

sys.path.insert(0, "/opt/trn_rl_repo")

import numpy as np

import concourse.bass as bass
import concourse.tile as tile
from concourse import bacc, mybir
from concourse.bass_utils import run_bass_kernel_spmd

B, R, C = 256, 1024, 1024
N_CORES = 8
BS = B // N_CORES  # planes per core

F32 = mybir.dt.float32
LT = mybir.AluOpType.is_lt
MULT = mybir.AluOpType.mult

_S1 = float(2.0 ** -100)
_S2 = float((2 ** 20 + 1) * 2.0 ** -49)
_NEG_FLT_MAX = -3.4028234663852886e38


def _runs(sorted_vals):
    """Contiguous runs [(start, len), ...] of a sorted unique int array."""
    runs = []
    for v in sorted_vals:
        if runs and v == runs[-1][0] + runs[-1][1]:
            runs[-1][1] += 1
        else:
            runs.append([int(v), 1])
    return [(s, n) for s, n in runs]


def _build(row_runs, have_patch):
    """Build the per-core Bacc graph, specialized on the masked-row layout."""
    nc = bacc.Bacc("TRN2", target_bir_lowering=False, debug=False, num_devices=N_CORES)
    x = nc.dram_tensor("x", [BS, R, C], F32, kind="ExternalInput").ap()
    colthr = nc.dram_tensor("colthr", [128, 8 * C], F32, kind="ExternalInput").ap()
    out = nc.dram_tensor("out", [BS, R, C], F32, kind="ExternalOutput").ap()

    # complement of masked rows -> bulk DRAM->DRAM copy ranges
    bulk_ranges = []
    pos = 0
    for s, n in row_runs:
        if s > pos:
            bulk_ranges.append((pos, s))
        pos = s + n
    if pos < R:
        bulk_ranges.append((pos, R))

    # segments: (plane, row_start, n_rows) with n_rows <= 128, packed into tiles
    segments = []
    for p in range(BS):
        for s, n in row_runs:
            o = 0
            while o < n:
                k = min(128, n - o)
                segments.append((p, s + o, k))
                o += k
    tiles = []  # list of lists of (plane, row_start, n_rows, part_ofs)
    cur, used = [], 0
    for p, s, n in segments:
        if used + n > 128:
            tiles.append(cur)
            cur, used = [], 0
        cur.append((p, s, n, used))
        used += n
    if cur:
        tiles.append(cur)

    # Fast path: a single run of masked rows (the spec's arange case) -> tile
    # partitions = masked rows, free dim = planes x cols ("p r c -> r p c"),
    # so the whole patch is a handful of big DMAs (3-dim APs).
    fast = (
        have_patch
        and len(row_runs) == 1
        and row_runs[0][1] in (16, 32, 64, 128)
        and BS % max(1, 128 // row_runs[0][1]) == 0
    )

    with tile.TileContext(nc) as tc:
        with tc.tile_pool(name="cmask", bufs=1) as cmask_pool, \
             tc.tile_pool(name="xin", bufs=2) as xin_pool, \
             tc.tile_pool(name="tmp", bufs=2) as tmp_pool:
            # Patch path rides the ACT HWDGE ring (nc.scalar) so it is not
            # FIFO-ordered behind the bulk copy on the SP ring (nc.sync).
            if fast:
                s0, nrow = row_runs[0]
                pper = 128 // nrow  # planes stacked along partitions
                cm = cmask_pool.tile([128, C], F32)
                nc.scalar.dma_start(cm[:], colthr[:, :C])
                for b in range(0, BS, pper):
                    t = xin_pool.tile([128, C], F32)
                    nc.scalar.dma_start(t[:], x[b:b + pper, s0:s0 + nrow, :])
                    mask = tmp_pool.tile([128, C], mybir.dt.int32, tag="mask")
                    nc.vector.tensor_tensor(mask[:], t[:], cm[:], LT)
                    muted = tmp_pool.tile([128, C], F32, tag="muted")
                    nc.vector.tensor_scalar(muted[:], t[:], _S1, _S2, MULT, MULT)
                    nc.vector.copy_predicated(t[:], mask[:], muted[:])
                    nc.scalar.dma_start(out[b:b + pper, s0:s0 + nrow, :], t[:])
            elif have_patch:
                cm = cmask_pool.tile([128, C], F32)
                nc.scalar.dma_start(cm[:], colthr[:, :])
                for segs in tiles:
                    t = xin_pool.tile([128, C], F32)
                    for p, s, n, ofs in segs:
                        nc.scalar.dma_start(t[ofs:ofs + n, :], x[p, s:s + n, :])
                    mask = tmp_pool.tile([128, C], mybir.dt.int32, tag="mask")
                    nc.vector.tensor_tensor(mask[:], t[:], cm[:], LT)
                    muted = tmp_pool.tile([128, C], F32, tag="muted")
                    nc.vector.tensor_scalar(muted[:], t[:], _S1, _S2, MULT, MULT)
                    nc.vector.copy_predicated(t[:], mask[:], muted[:])
                    for p, s, n, ofs in segs:
                        nc.scalar.dma_start(out[p, s:s + n, :], t[ofs:ofs + n, :])

            # Bulk: one contiguous DRAM->DRAM copy per plane per row-range.
            for p in range(BS):
                for r0, r1 in bulk_ranges:
                    nc.sync.dma_start(out[p, r0:r1, :], x[p, r0:r1, :])
    nc.compile()
    return nc


def kernel(x, rows, cols):
    x = np.ascontiguousarray(np.asarray(x, dtype=np.float32))
    rows = np.asarray(rows)
    cols = np.asarray(cols)
    assert x.shape == (B, R, C)

    rows_u = np.unique(rows.astype(np.int64))
    cols_u = np.unique(cols.astype(np.int64))
    have_patch = len(rows_u) > 0 and len(cols_u) > 0
    row_runs = _runs(rows_u) if have_patch else []

    colthr_vec = np.full((C,), _NEG_FLT_MAX, dtype=np.float64)
    if have_patch:
        colthr_vec[cols_u] = -0.5
    # replicated wide enough for the fast path's multi-plane free dim
    colthr = np.tile(colthr_vec.astype(np.float32)[None, :], (128, 8)).copy()

    nc = _build(row_runs, have_patch)

    in_maps = [
        {"x": x[i * BS:(i + 1) * BS], "colthr": colthr} for i in range(N_CORES)
    ]
    res = run_bass_kernel_spmd(nc, in_maps, core_ids=list(range(N_CORES)))
    kernel.last_result = res

    out = np.empty((B, R, C), dtype=np.float32)
    for i in range(N_CORES):
        out[i * BS:(i + 1) * BS] = res.results[i]["out"]
    return out


kernel.last_result = None


# revision 16
# speedup vs baseline: 1.8000x; 1.0703x over previous
"""Trainium2 Bass kernel for nn_ApproximationLayer (mute-MSB on a row/col grid).

The oracle (reference.py run on the neuron/axon jax backend of this container)
computes, for masked grid cells (r in rows, c in cols):

    y = RN_f32(x * c), c = (2^20+1) * 2^-149   if x < -0.5 (strictly)
    y = x                                       otherwise

(The jnp.frexp lowering on this backend value-converts x to int32 with
round-half-even saturation instead of bitcasting, so the exponent comes out
as 255 exactly when rint(x) <= -1, i.e. x < -0.5; exp2(-129.0) returns the
denormal 0x00400001 = (2^20+1)*2^-149 — one ulp above 2^-129; the where()
multiply is IEEE round-to-nearest-even including denormal results.)

Device strategy (8 NeuronCores, data-parallel over batch; 32 planes/core):
  - Only rows in `rows` can change. Bulk-copy all other rows DRAM->DRAM at
    full DMA rate (no compute engine touches ~94% of the data).
  - For masked rows, stream [128, 1024] f32 tiles through SBUF:
      mask  = x <  colthr        (colthr = -0.5 on selected cols, -FLT_MAX off)
      muted = (x * 2^-100) * ((2^20+1) * 2^-49)   # == RN(x*c), both steps on DVE
      x     = where(mask, muted, x)                # copy_predicated
    DVE denormal *results* are IEEE-exact on TRN2 (verified incl. ties);
    denormal *operands* are not, hence the two-step multiply.
"""
import sys

sys.path.insert(0, "/opt/trn_rl_repo")

import numpy as np

import concourse.bass as bass
import concourse.tile as tile
from concourse import bacc, mybir
from concourse.bass_utils import run_bass_kernel_spmd

B, R, C = 256, 1024, 1024
N_CORES = 8
BS = B // N_CORES  # planes per core

F32 = mybir.dt.float32
LT = mybir.AluOpType.is_lt
MULT = mybir.AluOpType.mult

_S1 = float(2.0 ** -100)
_S2 = float((2 ** 20 + 1) * 2.0 ** -49)
_NEG_FLT_MAX = -3.4028234663852886e38


def _runs(sorted_vals):
    """Contiguous runs [(start, len), ...] of a sorted unique int array."""
    runs = []
    for v in sorted_vals:
        if runs and v == runs[-1][0] + runs[-1][1]:
            runs[-1][1] += 1
        else:
            runs.append([int(v), 1])
    return [(s, n) for s, n in runs]


def _build(row_runs, have_patch):
    """Build the per-core Bacc graph, specialized on the masked-row layout."""
    nc = bacc.Bacc("TRN2", target_bir_lowering=False, debug=False, num_devices=N_CORES)
    x = nc.dram_tensor("x", [BS, R, C], F32, kind="ExternalInput").ap()
    colthr = nc.dram_tensor("colthr", [128, 8 * C], F32, kind="ExternalInput").ap()
    out = nc.dram_tensor("out", [BS, R, C], F32, kind="ExternalOutput").ap()

    # complement of masked rows -> bulk DRAM->DRAM copy ranges
    bulk_ranges = []
    pos = 0
    for s, n in row_runs:
        if s > pos:
            bulk_ranges.append((pos, s))
        pos = s + n
    if pos < R:
        bulk_ranges.append((pos, R))

    # segments: (plane, row_start, n_rows) with n_rows <= 128, packed into tiles
    segments = []
    for p in range(BS):
        for s, n in row_runs:
            o = 0
            while o < n:
                k = min(128, n - o)
                segments.append((p, s + o, k))
                o += k
    tiles = []  # list of lists of (plane, row_start, n_rows, part_ofs)
    cur, used = [], 0
    for p, s, n in segments:
        if used + n > 128:
            tiles.append(cur)
            cur, used = [], 0
        cur.append((p, s, n, used))
        used += n
    if cur:
        tiles.append(cur)

    # Fast path: a single run of masked rows (the spec's arange case) -> tile
    # partitions = masked rows, free dim = planes x cols ("p r c -> r p c"),
    # so the whole patch is a handful of big DMAs (3-dim APs).
    fast = (
        have_patch
        and len(row_runs) == 1
        and row_runs[0][1] in (16, 32, 64, 128)
        and BS % max(1, 128 // row_runs[0][1]) == 0
    )

    with tile.TileContext(nc) as tc:
        with tc.tile_pool(name="cmask", bufs=1) as cmask_pool, \
             tc.tile_pool(name="xin", bufs=2) as xin_pool, \
             tc.tile_pool(name="tmp", bufs=2) as tmp_pool:
            # Patch path rides the ACT HWDGE ring (nc.scalar) so it is not
            # FIFO-ordered behind the bulk copy on the SP ring (nc.sync).
            if fast:
                s0, nrow = row_runs[0]
                pper = 128 // nrow  # planes stacked along partitions
                cm = cmask_pool.tile([128, C], F32)
                nc.gpsimd.dma_start(cm[:], colthr[:, :C])
                for b in range(0, BS, pper):
                    t = xin_pool.tile([128, C], F32)
                    nc.gpsimd.dma_start(t[:], x[b:b + pper, s0:s0 + nrow, :])
                    mask = tmp_pool.tile([128, C], mybir.dt.int32, tag="mask")
                    nc.vector.tensor_tensor(mask[:], t[:], cm[:], LT)
                    muted = tmp_pool.tile([128, C], F32, tag="muted")
                    nc.vector.tensor_scalar(muted[:], t[:], _S1, _S2, MULT, MULT)
                    nc.vector.copy_predicated(t[:], mask[:], muted[:])
                    nc.gpsimd.dma_start(out[b:b + pper, s0:s0 + nrow, :], t[:])
            elif have_patch:
                cm = cmask_pool.tile([128, C], F32)
                nc.scalar.dma_start(cm[:], colthr[:, :])
                for segs in tiles:
                    t = xin_pool.tile([128, C], F32)
                    for p, s, n, ofs in segs:
                        nc.scalar.dma_start(t[ofs:ofs + n, :], x[p, s:s + n, :])
                    mask = tmp_pool.tile([128, C], mybir.dt.int32, tag="mask")
                    nc.vector.tensor_tensor(mask[:], t[:], cm[:], LT)
                    muted = tmp_pool.tile([128, C], F32, tag="muted")
                    nc.vector.tensor_scalar(muted[:], t[:], _S1, _S2, MULT, MULT)
                    nc.vector.copy_predicated(t[:], mask[:], muted[:])
                    for p, s, n, ofs in segs:
                        nc.scalar.dma_start(out[p, s:s + n, :], t[ofs:ofs + n, :])

            # Bulk: one contiguous DRAM->DRAM copy per plane per row-range.
            for p in range(BS):
                for r0, r1 in bulk_ranges:
                    nc.sync.dma_start(out[p, r0:r1, :], x[p, r0:r1, :])
    nc.compile()
    return nc


def kernel(x, rows, cols):
    x = np.ascontiguousarray(np.asarray(x, dtype=np.float32))
    rows = np.asarray(rows)
    cols = np.asarray(cols)
    assert x.shape == (B, R, C)

    rows_u = np.unique(rows.astype(np.int64))
    cols_u = np.unique(cols.astype(np.int64))
    have_patch = len(rows_u) > 0 and len(cols_u) > 0
    row_runs = _runs(rows_u) if have_patch else []

    colthr_vec = np.full((C,), _NEG_FLT_MAX, dtype=np.float64)
    if have_patch:
        colthr_vec[cols_u] = -0.5
    # replicated wide enough for the fast path's multi-plane free dim
    colthr = np.tile(colthr_vec.astype(np.float32)[None, :], (128, 8)).copy()

    nc = _build(row_runs, have_patch)

    in_maps = [
        {"x": x[i * BS:(i + 1) * BS], "colthr": colthr} for i in range(N_CORES)
    ]
    res = run_bass_kernel_spmd(nc, in_maps, core_ids=list(range(N_CORES)))
    kernel.last_result = res

    out = np.empty((B, R, C), dtype=np.float32)
    for i in range(N_CORES):
        out[i * BS:(i + 1) * BS] = res.results[i]["out"]
    return out


kernel.last_result = None


# revision 20
# speedup vs baseline: 2.8755x; 1.5975x over previous
"""Trainium2 Bass kernel for nn_ApproximationLayer (mute-MSB on a row/col grid).

The oracle (reference.py run on the neuron/axon jax backend of this container)
computes, for masked grid cells (r in rows, c in cols):

    y = RN_f32(x * c), c = (2^20+1) * 2^-149   if x < -0.5 (strictly)
    y = x                                       otherwise

(The jnp.frexp lowering on this backend value-converts x to int32 with
round-half-even saturation instead of bitcasting, so the exponent comes out
as 255 exactly when rint(x) <= -1, i.e. x < -0.5; exp2(-129.0) returns the
denormal 0x00400001 = (2^20+1)*2^-149 — one ulp above 2^-129; the where()
multiply is IEEE round-to-nearest-even including denormal results.)

Device strategy (8 NeuronCores, data-parallel over batch; 32 planes/core):
  - Only rows in `rows` can change. Bulk-copy all other rows DRAM->DRAM at
    full DMA rate (no compute engine touches ~94% of the data).
  - For masked rows, stream [128, 1024] f32 tiles through SBUF:
      mask  = x <  colthr        (colthr = -0.5 on selected cols, -FLT_MAX off)
      muted = (x * 2^-100) * ((2^20+1) * 2^-49)   # == RN(x*c), both steps on DVE
      x     = where(mask, muted, x)                # copy_predicated
    DVE denormal *results* are IEEE-exact on TRN2 (verified incl. ties);
    denormal *operands* are not, hence the two-step multiply.
"""
import sys

sys.path.insert(0, "/opt/trn_rl_repo")

import numpy as np

import concourse.bass as bass
import concourse.tile as tile
from concourse import bacc, mybir
from concourse.bass_utils import run_bass_kernel_spmd

B, R, C = 256, 1024, 1024
N_CORES = 8
BS = B // N_CORES  # planes per core

F32 = mybir.dt.float32
LT = mybir.AluOpType.is_lt
MULT = mybir.AluOpType.mult

_S1 = float(2.0 ** -100)
_S2 = float((2 ** 20 + 1) * 2.0 ** -49)
_NEG_FLT_MAX = -3.4028234663852886e38


def _runs(sorted_vals):
    """Contiguous runs [(start, len), ...] of a sorted unique int array."""
    runs = []
    for v in sorted_vals:
        if runs and v == runs[-1][0] + runs[-1][1]:
            runs[-1][1] += 1
        else:
            runs.append([int(v), 1])
    return [(s, n) for s, n in runs]


def _build(row_runs, have_patch):
    """Build the per-core Bacc graph, specialized on the masked-row layout."""
    nc = bacc.Bacc("TRN2", target_bir_lowering=False, debug=False, num_devices=N_CORES)
    x = nc.dram_tensor("x", [BS, R, C], F32, kind="ExternalInput").ap()
    colthr = nc.dram_tensor("colthr", [128, 8 * C], F32, kind="ExternalInput").ap()
    out = nc.dram_tensor("out", [BS, R, C], F32, kind="ExternalOutput").ap()

    # complement of masked rows -> bulk DRAM->DRAM copy ranges
    bulk_ranges = []
    pos = 0
    for s, n in row_runs:
        if s > pos:
            bulk_ranges.append((pos, s))
        pos = s + n
    if pos < R:
        bulk_ranges.append((pos, R))

    # segments: (plane, row_start, n_rows) with n_rows <= 128, packed into tiles
    segments = []
    for p in range(BS):
        for s, n in row_runs:
            o = 0
            while o < n:
                k = min(128, n - o)
                segments.append((p, s + o, k))
                o += k
    tiles = []  # list of lists of (plane, row_start, n_rows, part_ofs)
    cur, used = [], 0
    for p, s, n in segments:
        if used + n > 128:
            tiles.append(cur)
            cur, used = [], 0
        cur.append((p, s, n, used))
        used += n
    if cur:
        tiles.append(cur)

    # Fast path: a single run of masked rows (the spec's arange case) -> tile
    # partitions = masked rows, free dim = planes x cols ("p r c -> r p c"),
    # so the whole patch is a handful of big DMAs (3-dim APs).
    fast = (
        have_patch
        and len(row_runs) == 1
        and row_runs[0][1] in (16, 32, 64, 128)
        and BS % (512 // row_runs[0][1]) == 0
    )

    with tile.TileContext(nc) as tc:
        with tc.tile_pool(name="cmask", bufs=1) as cmask_pool, \
             tc.tile_pool(name="xin", bufs=2) as xin_pool, \
             tc.tile_pool(name="tmp", bufs=2) as tmp_pool:
            # Patch path rides the ACT HWDGE ring (nc.scalar) so it is not
            # FIFO-ordered behind the bulk copy on the SP ring (nc.sync).
            if fast:
                s0, nrow = row_runs[0]
                ppc = 512 // nrow   # planes per chunk: 128 partitions x 4 rows
                W = 4 * C           # 4 rows per partition, 16 KiB descriptors
                cm = cmask_pool.tile([128, W], F32)
                nc.gpsimd.dma_start(cm[:], colthr[:, :W])
                for b in range(0, BS, ppc):
                    t = xin_pool.tile([128, W], F32)
                    src = x[b:b + ppc, s0:s0 + nrow, :].rearrange(
                        "p (q w) c -> p q (w c)", w=4
                    )
                    nc.gpsimd.dma_start(t[:], src)
                    mask = tmp_pool.tile([128, W], mybir.dt.int32, tag="mask")
                    nc.vector.tensor_tensor(mask[:], t[:], cm[:], LT)
                    muted = tmp_pool.tile([128, W], F32, tag="muted")
                    nc.vector.tensor_scalar(muted[:], t[:], _S1, _S2, MULT, MULT)
                    nc.vector.copy_predicated(t[:], mask[:], muted[:])
                    dst = out[b:b + ppc, s0:s0 + nrow, :].rearrange(
                        "p (q w) c -> p q (w c)", w=4
                    )
                    nc.gpsimd.dma_start(dst, t[:])
            elif have_patch:
                cm = cmask_pool.tile([128, C], F32)
                nc.scalar.dma_start(cm[:], colthr[:, :])
                for segs in tiles:
                    t = xin_pool.tile([128, C], F32)
                    for p, s, n, ofs in segs:
                        nc.scalar.dma_start(t[ofs:ofs + n, :], x[p, s:s + n, :])
                    mask = tmp_pool.tile([128, C], mybir.dt.int32, tag="mask")
                    nc.vector.tensor_tensor(mask[:], t[:], cm[:], LT)
                    muted = tmp_pool.tile([128, C], F32, tag="muted")
                    nc.vector.tensor_scalar(muted[:], t[:], _S1, _S2, MULT, MULT)
                    nc.vector.copy_predicated(t[:], mask[:], muted[:])
                    for p, s, n, ofs in segs:
                        nc.scalar.dma_start(out[p, s:s + n, :], t[ofs:ofs + n, :])

            # Bulk: one contiguous DRAM->DRAM copy per plane per row-range.
            for p in range(BS):
                for r0, r1 in bulk_ranges:
                    nc.sync.dma_start(out[p, r0:r1, :], x[p, r0:r1, :])
    nc.compile()
    return nc


def kernel(x, rows, cols):
    x = np.ascontiguousarray(np.asarray(x, dtype=np.float32))
    rows = np.asarray(rows)
    cols = np.asarray(cols)
    assert x.shape == (B, R, C)

    rows_u = np.unique(rows.astype(np.int64))
    cols_u = np.unique(cols.astype(np.int64))
    have_patch = len(rows_u) > 0 and len(cols_u) > 0
    row_runs = _runs(rows_u) if have_patch else []

    colthr_vec = np.full((C,), _NEG_FLT_MAX, dtype=np.float64)
    if have_patch:
        colthr_vec[cols_u] = -0.5
    # replicated wide enough for the fast path's multi-plane free dim
    colthr = np.tile(colthr_vec.astype(np.float32)[None, :], (128, 8)).copy()

    nc = _build(row_runs, have_patch)

    in_maps = [
        {"x": x[i * BS:(i + 1) * BS], "colthr": colthr} for i in range(N_CORES)
    ]
    res = run_bass_kernel_spmd(nc, in_maps, core_ids=list(range(N_CORES)))
    kernel.last_result = res

    out = np.empty((B, R, C), dtype=np.float32)
    for i in range(N_CORES):
        out[i * BS:(i + 1) * BS] = res.results[i]["out"]
    return out


kernel.last_result = None
